# revision 1
# baseline (speedup 1.0000x reference)
"""Dynamic directional conv (depthwise 7x7, 4 rotated gaussian kernels mixed
per-pixel by an angle-MLP softmax) on 8 trn2 NeuronCores.

Strategy
--------
Data-parallel over batch B=8: one batch image per core.

Per core, the depthwise conv is computed as 28 banded matmuls per
4-channel group: for each direction d and kernel column kw, a banded
[128,128] matrix G_{d,kw} (the 7-tap H-conv for base_kernels[d][:,kw],
with reflect boundary folded in) contracts the H dimension on the tensor
engine; the W-shift for kw is a free-dim offset into the W-reflect-padded
image. The 7 kw-matmuls accumulate in PSUM. The per-pixel softmax weights
(computed on-device from angle_map via the 2-8-4 MLP) then mix the 4
directional results: multiplies on the vector engine (PSUM reads), adds
on the gpsimd engine (SBUF only) to keep DVE off the critical path.

Host prep (input marshaling): reflect-pad W, transpose x to (H, C, Wpad),
cast to fp16, and scatter base_kernels into the banded G stack. Matmuls
run in fp16 (fp32 PSUM accumulation), everything else in fp32.
"""

import math

import numpy as np

import concourse.bass as bass
import concourse.tile as tile
from concourse import bacc, mybir
from concourse.tile_rust import add_dep_helper
from concourse.bass_utils import run_bass_kernel_spmd

F16 = mybir.dt.float16
F32 = mybir.dt.float32

B, C, H, W = 8, 128, 128, 128
K = 7
PAD = K // 2
WP = W + 2 * PAD  # 134
NCHUNK = 8  # input DMA chunks (channel groups of 16)
CPC = C // NCHUNK  # channels per chunk
NCG = C // 4  # 4-channel matmul groups
N_CORES = 8

# consts layout: w1 (16) | b1 (8) | w2 (32) | b2 (4) | pi/2
IW1, IB1, IW2, IB2, IPI2 = 0, 16, 24, 56, 60
NCONST = 61

_cached_nc = None


def _build_nc():
    nc = bacc.Bacc("TRN2", target_bir_lowering=False, debug=False)
    xin_d = nc.dram_tensor("xin", [H, C, WP], F16, kind="ExternalInput")
    ang_d = nc.dram_tensor("angle", [H, W], F32, kind="ExternalInput")
    cst_d = nc.dram_tensor("consts", [NCONST], F32, kind="ExternalInput")
    g_d = nc.dram_tensor("gmat", [H, 28, H], F16, kind="ExternalInput")
    out_d = nc.dram_tensor("out", [C, H, W], F32, kind="ExternalOutput")

    with tile.TileContext(nc) as tc:
        with (
            tc.tile_pool(name="single", bufs=1) as single,
            tc.tile_pool(name="psum", bufs=1, space="PSUM") as psum,
            tc.tile_pool(name="accp", bufs=4) as accp,
            tc.tile_pool(name="tmpp", bufs=4) as tmpp,
        ):
            # ---- loads ----
            # angle first (tiny, unblocks the MLP), then x chunks on the SP
            # ring; gmat rides the ACT ring concurrently.
            at = single.tile([128, W], F32, tag="at")
            nc.sync.dma_start(out=at[:], in_=ang_d.ap())
            cb = single.tile([128, NCONST], F32, tag="cb")
            nc.gpsimd.dma_start(
                out=cb[:],
                in_=bass.AP(tensor=cst_d, offset=0, ap=[[0, 128], [1, NCONST]]),
            )
            gts = []
            for d in range(4):
                gtd = single.tile([128, K, H], F16, tag=f"gt{d}", name=f"gt{d}")
                gts.append(gtd)
            # G matrices ride the ACT ring; x chunks ride the SP ring.
            # Chain each transfer behind the previous one (SDMA engines
            # round-robin between queued DMAs at packet granularity, so an
            # unchained chunk 0 would be starved by co-draining later chunks).
            # channel ranges per DMA: two 8-wide leads, then 16-wide
            ranges = [(0, 8), (8, 16)] + [
                (c0, c0 + 16) for c0 in range(16, C, 16)
            ]
            xtiles = []
            xdmas = []
            for k, (c0, c1) in enumerate(ranges):
                t = single.tile([128, c1 - c0, WP], F16, tag=f"xw{k}", name=f"xw{k}")
                xi = nc.sync.dma_start(out=t[:], in_=xin_d.ap()[:, c0:c1, :])
                xtiles.append((c0, c1, t))
                xdmas.append(xi)
            gdmas = []
            for d in range(4):
                gi = nc.scalar.dma_start(
                    out=gts[d][:], in_=g_d.ap()[:, d * K : (d + 1) * K, :]
                )
                gdmas.append(gi)
            # one serial chain across both rings, ordered just-in-time:
            # gt0, x0a, gt1, gt2, gt3, x0b, x1, x2, ... (SDMA engines
            # round-robin queued DMAs at packet granularity, so unchained
            # early transfers would be starved by co-draining later ones)
            chain = [gdmas[0], xdmas[0], gdmas[1], gdmas[2], gdmas[3], xdmas[1]]
            chain += xdmas[2:]
            for a, b in zip(chain[1:], chain[:-1]):
                add_dep_helper(a.ins, b.ins, True, "serialize prefetch DMAs")

            def xview(cg):
                c0 = cg * 4
                for lo, hi, t in xtiles:
                    if lo <= c0 < hi:
                        return t, c0 - lo
                raise AssertionError

            # pre-allocated rotating tiles: avoids per-group pool release
            # machinery (~300 bookkeeping ops); Tile's WAW/reader tracking
            # still serializes reuse correctly.
            pbank = [
                psum.tile([128, 4 * W], F32, tag=f"mm{i}", name=f"mm{i}")
                for i in range(8)
            ]
            accb = [
                single.tile([128, 8, W], F32, tag=f"accb{i}", name=f"accb{i}")
                for i in range(4)
            ]
            tmpb = [
                single.tile([128, 4, W], F32, tag=f"tmpb{i}", name=f"tmpb{i}")
                for i in range(4)
            ]

            # ---- PE warmup: dummy matmuls so the HAM clock-gate is
            # warm (2.4 GHz) before the real stream starts ----
            wrm_l = single.tile([128, 128], F16, tag="wrm_l")
            wrm_r = single.tile([128, 512], F16, tag="wrm_r")
            nc.vector.memset(wrm_l[:], 0.0)
            nc.vector.memset(wrm_r[:], 0.0)
            for wi in range(12):
                nc.tensor.matmul(
                    pbank[wi % 8][:], wrm_l[:], wrm_r[:], start=True, stop=True
                )

            # ---- per-pixel mix weights: softmax(MLP(sin2a, cos2a)) ----
            sa = single.tile([128, W], F32, tag="sa")
            s2 = single.tile([128, W], F32, tag="s2")
            c2 = single.tile([128, W], F32, tag="c2")
            Act = mybir.ActivationFunctionType
            nc.scalar.activation(sa[:], at[:], Act.Sin)  # sin(a), a in [0,pi]
            # cos(a) = sin(pi/2 - a)
            nc.scalar.activation(
                c2[:], at[:], Act.Sin, bias=cb[:, IPI2 : IPI2 + 1], scale=-1.0
            )
            # sin(2a) = 2 sin(a) cos(a)
            nc.vector.tensor_mul(s2[:], sa[:], c2[:])
            nc.scalar.mul(out=s2[:], in_=s2[:], mul=2.0)
            # cos(2a) = 1 - 2 sin(a)^2
            nc.scalar.activation(c2[:], sa[:], Act.Square, scale=float(math.sqrt(2.0)))
            nc.vector.tensor_scalar(
                out=c2[:], in0=c2[:], scalar1=-1.0, scalar2=1.0,
                op0=mybir.AluOpType.mult, op1=mybir.AluOpType.add,
            )
            hall = single.tile([128, 8, W], F32, tag="hall")
            for j in range(8):
                nc.vector.tensor_scalar(
                    out=hall[:, j, :], in0=s2[:],
                    scalar1=cb[:, IW1 + 2 * j : IW1 + 2 * j + 1],
                    scalar2=cb[:, IB1 + j : IB1 + j + 1],
                    op0=mybir.AluOpType.mult, op1=mybir.AluOpType.add,
                )
                nc.vector.scalar_tensor_tensor(
                    out=hall[:, j, :], in0=c2[:],
                    scalar=cb[:, IW1 + 2 * j + 1 : IW1 + 2 * j + 2],
                    in1=hall[:, j, :],
                    op0=mybir.AluOpType.mult, op1=mybir.AluOpType.add,
                )
                nc.vector.tensor_scalar_max(
                    out=hall[:, j, :], in0=hall[:, j, :], scalar1=0.0
                )
            eall = single.tile([128, 4, W], F32, tag="eall")
            for d in range(4):
                nc.vector.tensor_scalar(
                    out=eall[:, d, :], in0=hall[:, 0, :],
                    scalar1=cb[:, IW2 + 8 * d : IW2 + 8 * d + 1],
                    scalar2=cb[:, IB2 + d : IB2 + d + 1],
                    op0=mybir.AluOpType.mult, op1=mybir.AluOpType.add,
                )
                for j in range(1, 8):
                    nc.vector.scalar_tensor_tensor(
                        out=eall[:, d, :], in0=hall[:, j, :],
                        scalar=cb[:, IW2 + 8 * d + j : IW2 + 8 * d + j + 1],
                        in1=eall[:, d, :],
                        op0=mybir.AluOpType.mult, op1=mybir.AluOpType.add,
                    )
                nc.scalar.activation(eall[:, d, :], eall[:, d, :], Act.Exp)
            ssum = single.tile([128, W], F32, tag="ssum")
            nc.vector.tensor_add(ssum[:], eall[:, 0, :], eall[:, 1, :])
            nc.vector.tensor_add(ssum[:], ssum[:], eall[:, 2, :])
            nc.vector.tensor_add(ssum[:], ssum[:], eall[:, 3, :])
            rs = single.tile([128, W], F32, tag="rs")
            nc.vector.reciprocal(rs[:], ssum[:])
            wall = single.tile([128, 4, W], F32, tag="wall")
            for d in range(4):
                nc.vector.tensor_mul(wall[:, d, :], eall[:, d, :], rs[:])

            # ---- banded conv + per-pixel mix ----
            # one 8-channel accumulator per cg pair -> one out DMA per pair
            gcount = 12
            tcount = 0
            for cg in range(NCG):
                xt, coff = xview(cg)
                ci = cg % 2
                if ci == 0:
                    acc = accb[(cg // 2) % 4]
                av = acc[:, ci * 4 : (ci + 1) * 4, :]
                for d in range(4):
                    p = pbank[gcount % 8]
                    gcount += 1
                    for kw in range(K):
                        rhs = xt[:, coff : coff + 4, kw : kw + W]
                        nc.tensor.matmul(
                            p[:],
                            gts[d][:, kw, :],
                            rhs,
                            start=(kw == 0),
                            stop=(kw == K - 1),
                        )
                    pv = p[:].rearrange("p (c w) -> p c w", c=4)
                    wdb = wall[:, d : d + 1, :].broadcast_to([128, 4, W])
                    if d == 0:
                        nc.vector.tensor_mul(av, pv, wdb)
                    else:
                        tmp = tmpb[tcount % 4]
                        tcount += 1
                        nc.vector.tensor_mul(tmp[:], pv, wdb)
                        if cg >= NCG - 2:
                            nc.vector.tensor_add(av, av, tmp[:])
                        else:
                            nc.gpsimd.tensor_add(av, av, tmp[:])
                if ci == 1:
                    cg0 = cg - 1
                    if cg == NCG - 1:
                        for q, eng in (
                            (0, nc.scalar), (1, nc.sync), (2, nc.scalar), (3, nc.sync)
                        ):
                            eng.dma_start(
                                out=out_d.ap()[
                                    cg0 * 4 + 2 * q : cg0 * 4 + 2 * q + 2
                                ].rearrange("c h w -> h c w"),
                                in_=acc[:, 2 * q : 2 * q + 2, :],
                            )
                    else:
                        nc.scalar.dma_start(
                            out=out_d.ap()[cg0 * 4 : cg0 * 4 + 8].rearrange(
                                "c h w -> h c w"
                            ),
                            in_=acc[:],
                        )

    nc.compile()
    return nc


def _build_gmat(base_kernels: np.ndarray) -> np.ndarray:
    """Banded H-conv matrices with reflect boundary: G[h, d*7+kw, m] so that
    (G[:,i,:].T @ img)[m, w] = sum_kh base[d,kh,kw] * img_reflectH[m+kh-3, w]."""
    g = np.zeros((H, 28, H), np.float32)
    m = np.arange(H)
    for d in range(4):
        for kw in range(K):
            for kh in range(K):
                i = m + kh - PAD
                i = np.where(i < 0, -i, i)
                i = np.where(i > H - 1, 2 * (H - 1) - i, i)
                np.add.at(g, (i, d * K + kw, m), base_kernels[d, kh, kw])
    return g.astype(np.float16)


# results of the last run_bass_kernel_spmd call (for test harnesses)
last_results = None


def kernel(x, angle_map, w1, b1, w2, b2, base_kernels):
    global _cached_nc, last_results
    x = np.asarray(x, np.float32)
    angle_map = np.asarray(angle_map, np.float32)
    consts = np.concatenate(
        [
            np.asarray(w1, np.float32).ravel(),
            np.asarray(b1, np.float32).ravel(),
            np.asarray(w2, np.float32).ravel(),
            np.asarray(b2, np.float32).ravel(),
            [math.pi / 2],
        ]
    ).astype(np.float32)
    gmat = _build_gmat(np.asarray(base_kernels, np.float32))

    # reflect-pad W, put H on the partition axis, cast to fp16
    xp = np.pad(x, ((0, 0), (0, 0), (0, 0), (PAD, PAD)), mode="reflect")
    xhcw = np.ascontiguousarray(xp.transpose(0, 2, 1, 3)).astype(np.float16)

    if _cached_nc is None:
        _cached_nc = _build_nc()
    nc = _cached_nc

    in_maps = [
        {"xin": xhcw[b], "angle": angle_map[b], "consts": consts, "gmat": gmat}
        for b in range(N_CORES)
    ]
    last_results = run_bass_kernel_spmd(nc, in_maps, core_ids=list(range(N_CORES)))
    return np.stack([last_results.results[b]["out"] for b in range(N_CORES)])

